# revision 10
# baseline (speedup 1.0000x reference)
"""Trainium2 Bass kernel for nn_NetSoNTopSIAMReg (adaptive-avg-pool + per-sample
top-k vote prefix sums).

Reference computation:
    x_sun = mean(maps, axis=(2,3))                        # [B, A]
    vote  = x_sun[:, None, :] * weight                    # [B, 1, A]
    sort |vote| desc; csum = cumsum(sorted_vote)
    x_topk[k] = csum[k-1] + avg   (k = 1..8)
    x_dense   = sum(vote) + avg
    x_son = [x_topk(1..8), x_dense]                       # [9, B, 1]

Sharding: data-parallel over batch B=32 across 8 cores (4 samples/core);
weight and avg_value replicated.

Per-core dataflow (81.9 MB shard):
  The naive layout (channels on partitions, spatial on free) makes every
  DMA descriptor strided 200704 B apart in DRAM; SWDGE cannot concat
  non-adjacent descriptors into streaming packets, so each descriptor pays
  a serial HBM round-trip per engine (~118 GB/s measured).  Instead each
  sample (102*50176 = 128*39984 floats) is streamed as fully CONTIGUOUS
  [128, 13328] tiles (measured 384 GB/s).  Since gcd(39984, 50176) = 784,
  reducing 784-float "units" (64 units per channel, 17 per chunk-row)
  never crosses a channel boundary.  Unit partials (26 KB/sample) bounce
  through DRAM to regroup partition-major unit order into per-channel
  groups, then a tiny second reduce + the top-8 selection loop finish on
  chip.
"""

import numpy as np

import concourse.bass as bass
import concourse.bacc as bacc
import concourse.mybir as mybir
from concourse import tile
from concourse.bass_utils import run_bass_kernel_spmd

B, A, H, W = 32, 102, 224, 224
S = H * W              # 50176 spatial elements per (b, a)
M = 8                  # cores
BS = B // M            # 4 samples per core
FLAT = A * S           # 5117952 = 128 * 39984 floats per sample
P = 128                # partitions for the streaming phase
UNIT = 784             # gcd(39984, 50176): channel-pure reduction unit
CH_UNITS = S // UNIT   # 64 units per channel
CU = 17                # units per partition per chunk
CHUNK_F = CU * UNIT    # 13328 floats per partition per chunk
NCHUNK = 3             # 3 * 17 = 51 units per partition = 39984 floats
CHUNK_FLAT = P * CHUNK_F          # 1705984 floats = 6.8 MB per DMA
UNITS_PER_SAMPLE = FLAT // UNIT   # 6528
UNITS_PER_CHUNK = P * CU          # 2176
TOPK = 8
NOUT = TOPK + 1        # 8 top-k prefix sums + 1 dense sum
INV_S = 1.0 / S
BIG = 1.0e30
FP = mybir.dt.float32
AX = mybir.AxisListType
ALU = mybir.AluOpType
ACTF = mybir.ActivationFunctionType


def build_program() -> bass.Bass:
    nc = bacc.Bacc("TRN2", debug=False)

    maps_in = nc.dram_tensor("maps", [BS, FLAT], FP, kind="ExternalInput")
    w_in = nc.dram_tensor("weight", [1, A], FP, kind="ExternalInput")
    avg_in = nc.dram_tensor("avg", [1, 1], FP, kind="ExternalInput")
    sun_out = nc.dram_tensor("x_sun", [BS, A], FP, kind="ExternalOutput")
    son_out = nc.dram_tensor("x_son", [BS, NOUT], FP, kind="ExternalOutput")
    scratch = nc.dram_tensor("unit_partials", [BS, UNITS_PER_SAMPLE], FP)

    with tile.TileContext(nc) as tc:
        with (
            tc.tile_pool(name="inp", bufs=3) as inp_pool,
            tc.tile_pool(name="acc", bufs=4) as acc_pool,
            tc.tile_pool(name="small", bufs=8) as small_pool,
            tc.tile_pool(name="p2", bufs=1) as p2_pool,
        ):
            # ---- Phase 1: contiguous streaming + unit partial sums ----
            for s in range(BS):
                for c in range(NCHUNK):
                    t = inp_pool.tile([P, CHUNK_F], FP, tag="inp")
                    src = maps_in[s, c * CHUNK_FLAT:(c + 1) * CHUNK_FLAT]
                    nc.gpsimd.dma_start(
                        out=t[:, :], in_=src.rearrange("(p f) -> p f", p=P, f=CHUNK_F)
                    )
                    acc = acc_pool.tile([P, CU], FP, tag="acc")
                    nc.vector.reduce_sum(
                        out=acc[:, :],
                        in_=t[:, :].rearrange("p (u f) -> p u f", u=CU, f=UNIT),
                        axis=AX.X,
                    )
                    # unit g = c*2176 + p*17 + u  ->  scratch[s, g] (flat unit order)
                    dst = scratch[s, c * UNITS_PER_CHUNK:(c + 1) * UNITS_PER_CHUNK]
                    nc.gpsimd.dma_start(
                        out=dst.rearrange("(p u) -> p u", p=P, u=CU), in_=acc[:, :]
                    )

            # ---- Regroup: channel a owns units [64a, 64a+64) of each sample ----
            rg = p2_pool.tile([A, BS * CH_UNITS], FP, tag="rg")
            nc.gpsimd.dma_start(
                out=rg[:, :],
                in_=scratch[:, :].rearrange("s (a v) -> a s v", a=A, v=CH_UNITS),
            )
            chans = p2_pool.tile([A, BS], FP, tag="chans")
            nc.vector.reduce_sum(
                out=chans[:, :],
                in_=rg[:, :].rearrange("a (s v) -> a s v", s=BS, v=CH_UNITS),
                axis=AX.X,
            )
            xsun_cols = p2_pool.tile([A, BS], FP, tag="xsun_cols")
            nc.scalar.activation(out=xsun_cols[:, :], in_=chans[:, :], func=ACTF.Copy, scale=INV_S)

            # transpose [A, BS] columns -> [BS, A] rows (SBUF->SBUF gather DMAs)
            xsun_t = p2_pool.tile([BS, A], FP, tag="xsun_t")
            for s in range(BS):
                nc.gpsimd.dma_start(out=xsun_t[s:s + 1, :], in_=xsun_cols[:, s:s + 1])

            # ---- Phase 2: votes + top-8 prefix sums (tiny) ----
            wt = p2_pool.tile([BS, A], FP, tag="wt")
            avt = p2_pool.tile([BS, 1], FP, tag="avt")
            for b in range(BS):
                nc.gpsimd.dma_start(out=wt[b:b + 1, :], in_=w_in[0:1, :])
                nc.gpsimd.dma_start(out=avt[b:b + 1, 0:1], in_=avg_in[0:1, 0:1])

            vote = p2_pool.tile([BS, A], FP, tag="vote")
            nc.vector.tensor_tensor(out=vote[:, :], in0=xsun_t[:, :], in1=wt[:, :], op=ALU.mult)
            absv = p2_pool.tile([BS, A], FP, tag="absv")
            nc.scalar.activation(out=absv[:, :], in_=vote[:, :], func=ACTF.Abs)

            # NOTE: tensor_tensor_reduce passes CoreSim + walrus but crashes the
            # device (NRT unrecoverable) — avoid it; use mul + reduce + add.
            csums = p2_pool.tile([BS, NOUT], FP, tag="csums")
            tot = small_pool.tile([BS, 1], FP, tag="tot")
            nc.vector.reduce_sum(out=tot[:, :], in_=vote[:, :], axis=AX.X)
            nc.vector.tensor_scalar(
                out=csums[:, TOPK:TOPK + 1], in0=tot[:, :], scalar1=avt[:, 0:1],
                scalar2=None, op0=ALU.add,
            )
            prev = avt[:, 0:1]  # running prefix sum, seeded with avg
            for k in range(TOPK):
                m = small_pool.tile([BS, 1], FP, tag="m")
                nc.vector.reduce_max(out=m[:, :], in_=absv[:, :], axis=AX.X)
                mask = small_pool.tile([BS, A], FP, tag="mask")
                nc.vector.tensor_scalar(
                    out=mask[:, :], in0=absv[:, :], scalar1=m[:, 0:1], scalar2=None,
                    op0=ALU.is_ge,
                )
                sel = small_pool.tile([BS, A], FP, tag="sel")
                nc.vector.tensor_tensor(out=sel[:, :], in0=vote[:, :], in1=mask[:, :], op=ALU.mult)
                step = small_pool.tile([BS, 1], FP, tag="step")
                nc.vector.reduce_sum(out=step[:, :], in_=sel[:, :], axis=AX.X)
                nc.vector.tensor_tensor(
                    out=csums[:, k:k + 1], in0=step[:, :], in1=prev, op=ALU.add,
                )
                prev = csums[:, k:k + 1]
                if k < TOPK - 1:
                    bigm = small_pool.tile([BS, A], FP, tag="bigm")
                    nc.vector.tensor_scalar_mul(out=bigm[:, :], in0=mask[:, :], scalar1=BIG)
                    nc.vector.tensor_sub(out=absv[:, :], in0=absv[:, :], in1=bigm[:, :])

            nc.gpsimd.dma_start(out=sun_out[:, :], in_=xsun_t[:, :])
            nc.gpsimd.dma_start(out=son_out[:, :], in_=csums[:, :])

    nc.compile()
    return nc


def _install_axon_ntff_shim():
    """bass_utils' trace=True path under axon imports ``antenv.axon_hooks``,
    which this image lacks; synthesize the module so NTFF profiling works.
    Degrades to trace-disabled on any failure."""
    import sys
    import types

    if "antenv.axon_hooks" in sys.modules:
        return
    try:
        from trn_agent_boot.trn_boot import _ntff_profile_via_ctypes

        hook = _ntff_profile_via_ctypes("/opt/axon/libaxon_pjrt.so")
        mod = types.ModuleType("antenv.axon_hooks")
        mod._hook = hook
        mod.get_axon_ntff_profile_hook = lambda: mod._hook

        def _set(h):
            mod._hook = h

        mod.set_axon_ntff_profile_hook = _set
        sys.modules["antenv.axon_hooks"] = mod
    except Exception:
        import os

        os.environ.setdefault("BASS_NEVER_TRACE", "1")


_NC_CACHE: list = []
LAST_RESULTS = None  # BassKernelResults of the most recent kernel() call


def _get_nc() -> bass.Bass:
    if not _NC_CACHE:
        _NC_CACHE.append(build_program())
    return _NC_CACHE[0]


def kernel(maps, weight, avg_value):
    maps = np.ascontiguousarray(np.asarray(maps, dtype=np.float32)).reshape(B, FLAT)
    weight = np.ascontiguousarray(np.asarray(weight, dtype=np.float32)).reshape(1, A)
    avg = np.asarray(avg_value, dtype=np.float32).reshape(1, 1)

    _install_axon_ntff_shim()
    nc = _get_nc()
    in_maps = [
        {"maps": maps[i * BS:(i + 1) * BS], "weight": weight, "avg": avg}
        for i in range(M)
    ]
    res = run_bass_kernel_spmd(nc, in_maps, core_ids=list(range(M)))
    global LAST_RESULTS
    LAST_RESULTS = res
    outs = res.results

    x_sun = np.concatenate([outs[i]["x_sun"] for i in range(M)], axis=0)
    son = np.concatenate([outs[i]["x_son"] for i in range(M)], axis=0)   # [B, 9]
    x_son = np.ascontiguousarray(son.T)[:, :, None]                      # [9, B, 1]
    return x_sun.astype(np.float32), x_son.astype(np.float32)
